# revision 22
# baseline (speedup 1.0000x reference)
"""Trainium2 Bass kernel: 4-layer ChebConv(K=3) GNN + MLP readout + scatter-mean.

Sharding: nodes (and their incident edges, partitioned by TARGET node) are
split across 8 cores. Each propagate is gather(full source tensor) ->
scale-by-wn -> scatter-add(local accumulator). The full source tensor is
produced by AllGather of per-core blocks. ChebConv weights / readout MLP are
replicated.

Math restructure per layer (folded Chebyshev recurrence):
    out = Tx0@(W0-W2) + b + Tx1@W1 + prop(Tx1)@(2*W2),  Tx1 = prop(Tx0)
and prop(H@W) = prop(H)@W, so the second propagate directly consumes
z = Tx1@(2*W2) and scatter-adds into the partially-computed output.
"""

import numpy as np

from concourse import bass, library_config, masks, mybir, tile
from concourse.bass_utils import run_bass_kernel_spmd

F32 = mybir.dt.float32
I16 = mybir.dt.int16
ADD = mybir.AluOpType.add
MULT = mybir.AluOpType.mult


def _rup(a, b):
    return -(-a // b) * b


# scatter/gather ucode lane swizzle: within a 128-token chunk, the token at
# partition p is handled by DMA ring k(p) = 2*((p%32)//4) + p//64; ring k owns
# partitions FIRST[k] + OFF[b], processed in ascending b order.
FIRST = np.array([0, 64, 4, 68, 8, 72, 12, 76, 16, 80, 20, 84, 24, 88, 28, 92])
OFF = np.array([0, 1, 2, 3, 32, 33, 34, 35])


def _schedule(ls, tl, M):
    """Assign each token an engine-slot so that all adds to one target row go
    through one DMA ring (engine = tloc%16) with >= M descriptors between
    consecutive same-row adds (RMW hazard window). Targets get a fixed rank
    (descending degree); occurrence r of a target sits at base[r] + rank,
    where base[r+1] - base[r] = max(n_r, M), n_r = #targets with degree > r.
    Input must be sorted by (ls, tl). Returns (engine_pos, max ring length).
    """
    n = len(ls)
    key = ls.astype(np.int64) * (1 << 32) + tl
    uk, first_i, cnt_t = np.unique(key, return_index=True, return_counts=True)
    eng_u = (uk >> 32).astype(np.int64)
    tl_u = uk & 0xFFFFFFFF
    order_u = np.lexsort((tl_u, -cnt_t, eng_u))
    eng_sorted = eng_u[order_u]
    engcnt = np.bincount(eng_sorted, minlength=16)
    engstart = np.concatenate([[0], np.cumsum(engcnt)])[:16]
    rank_u = np.empty(len(uk), np.int64)
    rank_u[order_u] = np.arange(len(uk)) - engstart[eng_sorted]
    inv = np.searchsorted(uk, key)
    r_tok = np.arange(n) - first_i[inv]
    jpos = np.empty(n, np.int64)
    need = 0
    for k in range(16):
        selu = eng_u == k
        if not selu.any():
            continue
        c_k = cnt_t[selu]
        dm = int(c_k.max())
        hist = np.bincount(c_k, minlength=dm + 1)
        nr = len(c_k) - np.cumsum(hist)[:dm]
        base = np.concatenate([[0], np.cumsum(np.maximum(nr, M))])
        need = max(need, int(base[-1]))
        m = ls == k
        jpos[m] = base[r_tok[m]] + rank_u[inv[m]]
    return jpos, need


class Cfg:
    def __init__(self, N=100000, E=1600000, G=64, L=4, C=8,
                 CH=896, ALIGN=896, BUCKET=16384, MSP=64):
        self.N, self.E, self.G, self.L, self.C = N, E, G, L, C
        self.D = 64
        self.CH, self.ALIGN, self.BUCKET = CH, ALIGN, BUCKET
        self.MSP = MSP
        assert N % C == 0
        self.B = N // C                 # real nodes per core
        self.NT = _rup(self.B, 128) // 128   # 128-row tiles per core
        self.BP = self.NT * 128         # padded nodes per core
        self.NP = C * self.BP           # padded full node count
        self.NB = _rup(self.NP, BUCKET) // BUCKET
        assert ALIGN % 128 == 0 and CH % ALIGN == 0
        assert BUCKET <= 32768
        # per-queue DMA descriptor ring holds 128 entries; gather needs
        # n/16+1 per direction, scatter-add 2*(n/16)+1 m2s
        assert CH <= 1920
        self.SC = 896                   # max tokens per scatter instruction
        assert MSP >= self.SC // 16     # same-row adds never share an instr
        self.ST = 7 if self.NT % 7 == 0 else (2 if self.NT % 2 == 0 else 1)
        self.NS = self.NT // self.ST


def _preprocess(cfg, x, edge_index, edge_weight, batch,
                w_layers, b_layers, wr1, br1, wr2, br2):
    C, B, BP, NP = cfg.C, cfg.B, cfg.BP, cfg.NP
    BUCKET, NB, ALIGN, CH = cfg.BUCKET, cfg.NB, cfg.ALIGN, cfg.CH
    G, L, D = cfg.G, cfg.L, cfg.D

    x = np.asarray(x, np.float32)
    edge_index = np.asarray(edge_index, np.int64)
    ew = np.asarray(edge_weight, np.float32)
    batch = np.asarray(batch, np.int64)
    w_layers = np.asarray(w_layers, np.float32)
    b_layers = np.asarray(b_layers, np.float32)
    wr1 = np.asarray(wr1, np.float32)
    br1 = np.asarray(br1, np.float32)
    wr2 = np.asarray(wr2, np.float32)
    br2 = np.asarray(br2, np.float32)

    src, tgt = edge_index[0], edge_index[1]
    deg = np.bincount(src, weights=ew, minlength=cfg.N).astype(np.float32)
    dis = np.where(deg > 0, 1.0 / np.sqrt(np.where(deg > 0, deg, 1.0)), 0.0)
    dis = dis.astype(np.float32)
    wn = (-dis[src] * ew * dis[tgt]).astype(np.float32)

    psrc = (src // B) * BP + (src % B)      # padded source row
    core = tgt // B
    tloc = tgt % B
    pbkt = psrc // BUCKET
    lane = tloc % 16

    order = np.lexsort((tloc, lane, pbkt, core))
    core_s, pbkt_s = core[order], pbkt[order]
    lane_s, tloc_s = lane[order], tloc[order]
    psrc_s, wn_s = psrc[order], wn[order]

    # segment boundaries per (core, bucket)
    key = core_s * NB + pbkt_s
    bounds = np.searchsorted(key, np.arange(C * NB + 1))

    # per-(core,bucket) race-safe engine schedules; per-bucket shared segment
    # size = max over cores of 16 * longest ring
    sched = {}
    lmax = np.zeros((C, NB), np.int64)
    for c in range(C):
        for b in range(NB):
            lo, hi = bounds[c * NB + b], bounds[c * NB + b + 1]
            if hi > lo:
                jpos, need = _schedule(lane_s[lo:hi], tloc_s[lo:hi], cfg.MSP)
                sched[(c, b)] = jpos
                lmax[c, b] = need
    S = [int(_rup(16 * int(lmax[:, b].max()), ALIGN)) if lmax[:, b].max() > 0
         else 0 for b in range(NB)]
    seg0 = np.concatenate([[0], np.cumsum(S)])
    TOT = int(seg0[-1])

    chunks = []  # (bucket, token_start, ntok)
    for b in range(NB):
        off = 0
        while off < S[b]:
            n = min(CH, S[b] - off)
            chunks.append((b, int(seg0[b] + off), int(n)))
            off += n

    # token streams per core. Pad tokens (wn=0) still RMW their target row,
    # so ring k's pads get a private dummy row BP+k that no real add touches.
    pp = np.arange(TOT) % 128
    keng = 2 * ((pp % 32) // 4) + (pp // 64)
    tgt_default = (BP + keng).astype(np.int16)
    maps = []
    xp = np.zeros((NP, D), np.float32)
    for c in range(C):
        xp[c * BP:c * BP + B] = x[c * B:(c + 1) * B]

    wm = np.zeros((D, L * 3 * D), np.float32)
    for l in range(L):
        wm[:, l * 192:l * 192 + 64] = w_layers[l, 0] - w_layers[l, 2]
        wm[:, l * 192 + 64:l * 192 + 128] = w_layers[l, 1]
        wm[:, l * 192 + 128:l * 192 + 192] = 2.0 * w_layers[l, 2]
    brep = np.tile(b_layers.reshape(1, L * D), (128, 1)).astype(np.float32)
    br1rep = np.tile(br1.reshape(1, -1), (128, 1)).astype(np.float32)
    zblk = np.zeros((BP, D), np.float32)

    for c in range(C):
        s16 = np.zeros(TOT, np.int16)
        t16 = tgt_default.copy()
        wnf = np.zeros(TOT, np.float32)
        for b in range(NB):
            lo, hi = bounds[c * NB + b], bounds[c * NB + b + 1]
            if hi <= lo:
                continue
            ls = lane_s[lo:hi]
            jpos = sched[(c, b)]
            tok = seg0[b] + (jpos // 8) * 128 + FIRST[ls] + OFF[jpos % 8]
            s16[tok] = (psrc_s[lo:hi] - b * BUCKET).astype(np.int16)
            t16[tok] = tloc_s[lo:hi].astype(np.int16)
            wnf[tok] = wn_s[lo:hi]

        srcidx = np.tile(np.ascontiguousarray(s16.reshape(-1, 16).T), (8, 1))
        tgtidx = np.tile(np.ascontiguousarray(t16.reshape(-1, 16).T), (8, 1))
        wnt = np.ascontiguousarray(wnf.reshape(-1, 128).T)

        pool = np.zeros((BP, G), np.float32)
        bb = batch[c * B:(c + 1) * B]
        pool[np.arange(B), bb] = 1.0

        maps.append({
            "xp": xp,
            "xloc": np.ascontiguousarray(xp[c * BP:(c + 1) * BP]),
            "srcidx": srcidx, "tgtidx": tgtidx, "wnt": wnt,
            "wmat": wm, "brep": brep, "wr1": wr1, "br1rep": br1rep,
            "wr2": wr2, "pool": pool, "zblk": zblk,
        })

    cnt_g = np.bincount(batch, minlength=G).astype(np.float32)[:G]
    return dict(TOT=TOT, chunks=chunks, maps=maps, cnt=cnt_g, br2=float(br2[0]))


def _build(cfg, TOT, chunks):
    C, L, G, D = cfg.C, cfg.L, cfg.G, cfg.D
    NT, BP, NP, ST, NS = cfg.NT, cfg.BP, cfg.NP, cfg.ST, cfg.NS
    CH, BUCKET, SC = cfg.CH, cfg.BUCKET, cfg.SC

    nc = bass.Bass()
    xp_p = nc.declare_dram_parameter("xp", [NP, D], F32, isOutput=False)
    xloc_p = nc.declare_dram_parameter("xloc", [BP, D], F32, isOutput=False)
    srcidx_p = nc.declare_dram_parameter("srcidx", [128, TOT // 16], I16, isOutput=False)
    tgtidx_p = nc.declare_dram_parameter("tgtidx", [128, TOT // 16], I16, isOutput=False)
    wnt_p = nc.declare_dram_parameter("wnt", [128, TOT // 128], F32, isOutput=False)
    wmat_p = nc.declare_dram_parameter("wmat", [D, L * 3 * D], F32, isOutput=False)
    brep_p = nc.declare_dram_parameter("brep", [128, L * D], F32, isOutput=False)
    wr1_p = nc.declare_dram_parameter("wr1", [D, 32], F32, isOutput=False)
    br1_p = nc.declare_dram_parameter("br1rep", [128, 32], F32, isOutput=False)
    wr2_p = nc.declare_dram_parameter("wr2", [32, 1], F32, isOutput=False)
    pool_p = nc.declare_dram_parameter("pool", [BP, G], F32, isOutput=False)
    zblk_p = nc.declare_dram_parameter("zblk", [BP, D], F32, isOutput=False)
    out_p = nc.declare_dram_parameter("opart", [G, 1], F32, isOutput=True)

    # scatter destinations carry 16 extra dummy rows (one per DMA ring) that
    # absorb pad-token RMWs; only rows [:BP] are ever read back
    acc = [nc.dram_tensor(f"acc{l}", [BP + 16, D], F32) for l in range(L)]
    yacc = [nc.dram_tensor(f"yacc{l}", [BP + 16, D], F32) for l in range(L)]
    zloc = [nc.dram_tensor(f"zloc{l}", [BP, D], F32) for l in range(L)]
    zfull = [nc.dram_tensor(f"zfull{l}", [NP, D], F32, addr_space="Shared")
             for l in range(L)]
    hy = [nc.dram_tensor(f"hy{l}", [NP, D], F32, addr_space="Shared")
          for l in range(L - 1)]
    RG = [list(range(C))]

    with tile.TileContext(nc) as tc:
        with tc.tile_pool(name="const", bufs=1) as const:
            srcidx_s = const.tile([128, TOT // 16], I16)
            tgtidx_s = const.tile([128, TOT // 16], I16)
            wnt_s = const.tile([128, TOT // 128], F32)
            wmat_s = const.tile([D, L * 3 * D], F32)
            brep_s = const.tile([128, L * D], F32)
            wr1_s = const.tile([D, 32], F32)
            br1_s = const.tile([128, 32], F32)
            wr2_s = const.tile([32, 1], F32)
            ident = const.tile([128, 128], F32)

            nc.sync.dma_start(srcidx_s[:], srcidx_p[:])
            nc.sync.dma_start(tgtidx_s[:], tgtidx_p[:])
            nc.sync.dma_start(wnt_s[:], wnt_p[:])
            nc.sync.dma_start(wmat_s[:], wmat_p[:])
            nc.sync.dma_start(brep_s[:], brep_p[:])
            nc.sync.dma_start(wr1_s[:], wr1_p[:])
            nc.sync.dma_start(br1_s[:], br1_p[:])
            nc.sync.dma_start(wr2_s[:], wr2_p[:])
            masks.make_identity(nc, ident[:])
            nc.gpsimd.load_library(library_config.mlp)
            for l in range(L):
                nc.sync.dma_start(acc[l][:BP, :], zblk_p[:])

            sizes = set()
            for (_, _, n) in chunks:
                sizes.add(n)
                for off in range(0, n, SC):
                    sizes.add(min(SC, n - off))
            nreg = {}
            for nval in sorted(sizes):
                r = nc.gpsimd.alloc_register(f"nreg{nval}")
                nc.gpsimd.reg_mov(r, nval)
                nreg[nval] = r

            def prop(srch, dsth, gpool):
                for (b, t0, n) in chunks:
                    g = gpool.tile([128, CH // 128, D], F32)
                    lo = b * BUCKET
                    hi = min(lo + BUCKET, NP)
                    nc.gpsimd.dma_gather(
                        g[:, :n // 128, :], srch[lo:hi, :],
                        srcidx_s[:, t0 // 16:(t0 + n) // 16], n, nreg[n], D)
                    wb = (wnt_s[:, t0 // 128:(t0 + n) // 128]
                          .unsqueeze(2).to_broadcast((128, n // 128, D)))
                    nc.vector.tensor_tensor(
                        g[:, :n // 128, :], g[:, :n // 128, :], wb, MULT)
                    # same-row adds within one scatter instruction race (the
                    # read-modify-write reads all issue before writes land),
                    # so each row appears at most once per <=896-token scatter
                    # (schedule spacing MSP > 896/16); consecutive scatters to
                    # one dst are WAW-serialized, which HW honors exactly
                    for off in range(0, n, SC):
                        m = min(SC, n - off)
                        nc.gpsimd.dma_scatter_add(
                            dsth[:], g[:, off // 128:(off + m) // 128, :],
                            tgtidx_s[:, (t0 + off) // 16:(t0 + off + m) // 16],
                            m, nreg[m], D)

            with tc.tile_pool(name="gath", bufs=3) as gpool, \
                 tc.tile_pool(name="ld", bufs=4) as ldp, \
                 tc.tile_pool(name="tsb", bufs=4) as tsb, \
                 tc.tile_pool(name="stg", bufs=4) as stg, \
                 tc.psum_pool(name="tp", bufs=2) as tp, \
                 tc.psum_pool(name="yp", bufs=2) as ypp, \
                 tc.psum_pool(name="zp", bufs=2) as zpp:

                for l in range(L):
                    f0 = xp_p if l == 0 else hy[l - 1]
                    tx0 = xloc_p if l == 0 else yacc[l - 1]
                    prop(f0, acc[l], gpool)

                    tx0r = tx0[:BP, :].rearrange("(n p) f -> p n f", p=128)
                    accr = acc[l][:BP, :].rearrange("(n p) f -> p n f", p=128)
                    yaccr = yacc[l][:BP, :].rearrange("(n p) f -> p n f", p=128)
                    zlocr = zloc[l][:].rearrange("(n p) f -> p n f", p=128)
                    for s in range(NS):
                        xt = ldp.tile([128, ST, D], F32)
                        at = ldp.tile([128, ST, D], F32)
                        nc.sync.dma_start(xt[:], tx0r[:, s * ST:(s + 1) * ST, :])
                        nc.sync.dma_start(at[:], accr[:, s * ST:(s + 1) * ST, :])
                        yb = stg.tile([128, ST, D], F32)
                        zb = stg.tile([128, ST, D], F32)
                        for j in range(ST):
                            xTp = tp.tile([D, 128], F32)
                            nc.tensor.transpose(xTp[:], xt[:, j, :], ident[:])
                            aTp = tp.tile([D, 128], F32)
                            nc.tensor.transpose(aTp[:], at[:, j, :], ident[:])
                            xT = tsb.tile([D, 128], F32)
                            nc.vector.tensor_copy(xT[:], xTp[:])
                            aT = tsb.tile([D, 128], F32)
                            nc.scalar.copy(aT[:], aTp[:])
                            ypt = ypp.tile([128, D], F32)
                            nc.tensor.matmul(
                                ypt[:], xT[:], wmat_s[:, l * 192:l * 192 + 64],
                                start=True, stop=False)
                            nc.tensor.matmul(
                                ypt[:], aT[:], wmat_s[:, l * 192 + 64:l * 192 + 128],
                                start=False, stop=True)
                            zpt = zpp.tile([128, D], F32)
                            nc.tensor.matmul(
                                zpt[:], aT[:], wmat_s[:, l * 192 + 128:l * 192 + 192],
                                start=True, stop=True)
                            nc.vector.tensor_tensor(
                                yb[:, j, :], ypt[:], brep_s[:, l * D:(l + 1) * D], ADD)
                            nc.scalar.copy(zb[:, j, :], zpt[:])
                        nc.sync.dma_start(yaccr[:, s * ST:(s + 1) * ST, :], yb[:])
                        nc.sync.dma_start(zlocr[:, s * ST:(s + 1) * ST, :], zb[:])

                    nc.gpsimd.collective_compute(
                        "AllGather", mybir.AluOpType.bypass, RG,
                        ins=[zloc[l][:]], outs=[zfull[l][:]])
                    prop(zfull[l], yacc[l], gpool)
                    if l < L - 1:
                        nc.gpsimd.collective_compute(
                            "AllGather", mybir.AluOpType.bypass, RG,
                            ins=[yacc[l][:BP, :]], outs=[hy[l][:]])

            # readout: h = relu(y@wr1+br1); qT = h^T @ pool; o = q @ wr2
            with tc.tile_pool(name="rld", bufs=4) as rld, \
                 tc.tile_pool(name="rsb", bufs=4) as rsb, \
                 tc.tile_pool(name="qac", bufs=1) as qac, \
                 tc.psum_pool(name="rtp", bufs=2) as rtp, \
                 tc.psum_pool(name="rhp", bufs=2) as rhp, \
                 tc.psum_pool(name="rqp", bufs=2) as rqp:
                q_acc = qac.tile([32, G], F32)
                nc.vector.memset(q_acc[:], 0.0)
                y3r = yacc[L - 1][:BP, :].rearrange("(n p) f -> p n f", p=128)
                poolr = pool_p[:].rearrange("(n p) g -> p n g", p=128)
                for s in range(NS):
                    yt = rld.tile([128, ST, D], F32)
                    pt = rld.tile([128, ST, G], F32)
                    nc.sync.dma_start(yt[:], y3r[:, s * ST:(s + 1) * ST, :])
                    nc.sync.dma_start(pt[:], poolr[:, s * ST:(s + 1) * ST, :])
                    for j in range(ST):
                        yTp = rtp.tile([D, 128], F32)
                        nc.tensor.transpose(yTp[:], yt[:, j, :], ident[:])
                        yT = rsb.tile([D, 128], F32)
                        nc.vector.tensor_copy(yT[:], yTp[:])
                        hp = rhp.tile([128, 32], F32)
                        nc.tensor.matmul(hp[:], yT[:], wr1_s[:],
                                         start=True, stop=True)
                        hs = rsb.tile([128, 32], F32)
                        nc.vector.tensor_tensor(hs[:], hp[:], br1_s[:], ADD)
                        nc.vector.tensor_scalar_max(hs[:], hs[:], 0.0)
                        qd = rqp.tile([32, G], F32)
                        nc.tensor.matmul(qd[:], hs[:], pt[:, j, :],
                                         start=True, stop=True)
                        nc.vector.tensor_tensor(q_acc[:], q_acc[:], qd[:], ADD)
                qs = rsb.tile([32, G], F32)
                nc.vector.tensor_copy(qs[:], q_acc[:])
                op_t = rhp.tile([G, 1], F32)
                nc.tensor.matmul(op_t[:], qs[:], wr2_s[:], start=True, stop=True)
                osb = rsb.tile([G, 1], F32)
                nc.vector.tensor_copy(osb[:], op_t[:])
                nc.sync.dma_start(out_p[:], osb[:])

    # HW allows at most 1 sync wait per instruction — split excess waits
    # onto InstEventSemaphore (the Bacc pass; tile doesn't enforce this)
    mybir._bass_rust.generate_event_semaphores(nc)
    # populate .instr bytes for extended-inst InstISA subclasses
    # (DMAGatherAnt etc.) — raw Bass skips this Bacc pass and the NEFF
    # compiler rejects empty .instr with "ISA wrong length"
    from concourse.library_overlay import lower_extended_insts

    lower_extended_insts(nc)
    return nc


def kernel(**inputs):
    cfg = Cfg()
    pre = _preprocess(cfg, **inputs)
    nc = _build(cfg, pre["TOT"], pre["chunks"])
    res = run_bass_kernel_spmd(nc, pre["maps"], list(range(cfg.C))).results
    osum = np.zeros(cfg.G, np.float64)
    for c in range(cfg.C):
        osum += res[c]["opart"].reshape(cfg.G).astype(np.float64)
    cnt = pre["cnt"]
    out = (osum.astype(np.float32) + pre["br2"] * cnt) / np.maximum(cnt, 1.0)
    return out.reshape(cfg.G, 1).astype(np.float32)
